# revision 21
# baseline (speedup 1.0000x reference)
"""Trainium2 Bass kernel for EnhancedSNN (2-layer LIF spiking net, 50 steps).

Math:
    cur1 = x @ w1.T + b1                      (loop-invariant)
    m_0 = cur1;  m_t = b*m_{t-1} + cur1 - spk_{t-1};  spk_t = (m_t > 1)
    cur2_t = spk1_t @ w2.T + b2               (layer 2 analogous, cur2 varies)

Transform (layer 1): K = cur1/(1-b), z = m - K  =>  z_t = b*z_{t-1} - spk_{t-1},
    spk_t = (z_t > thr),  thr = 1 - K = 1 + z_0/b,  z_0 = -(b/(1-b)) * cur1.
This removes the per-step "+cur1" elementwise pass entirely.

Engine split per step (state lives in PSUM, 4 banks of [128,512]):
    We keep the UNDECAYED state w_t = z_t / b^t in PSUM. Then
        spk_t = (z_t > thr) = ((w_t * b^t) > thr)     [one fused DVE STT op]
        w_{t+1} = w_t - b^{-(t+1)} * spk_t            [PE accumulating matmul,
                  lhsT = hi/lo bf16 split of -b^{-(t+1)} * I, exact to ~2^-17]
    No ScalarE pass and no in-place PSUM RMW by ACT/DVE (which faults the
    exec unit on HW); only TensorE accumulates into PSUM. has_written bits
    stay set from fc1 since only TensorE touches them.
    TensorE also: cur2 = spk @ w2.T + b2  (spk slices as fp16 weights, hi/lo)

Sharding: data-parallel over batch, 8 cores x 256 batch rows. Weights replicated.
Device layout: [neuron on partitions, batch on free]; spikes emitted as fp16
(0/1 exact) and transposed/cast to f32 on host.
"""

import sys

sys.path.insert(0, "/opt/trn_rl_repo")

import numpy as np
import ml_dtypes

import concourse.bass as bass
import concourse.bacc as bacc
import concourse.mybir as mybir
from concourse.tile import TileContext
from concourse import bass_utils

F32 = mybir.dt.float32
H16 = mybir.dt.float16
NP_H16 = np.float16

T = 50
BETA = 0.9
B, NI, NH, NO = 2048, 1024, 1024, 10
NCORES = 8
BC = B // NCORES  # 256 batch rows per core
NHC = NH // 128  # 8 neuron chunks
NBANK = 4  # z PSUM banks, each [128, 512] f32 covering two neuron chunks

# Set by test harness; kernel() stores BassKernelResults here.
TRACE = False
LAST_RESULTS = None

_CACHED = {}  # repeats -> built program


def _split_hilo(a32):
    hi = a32.astype(NP_H16)
    lo = (a32 - hi.astype(np.float32)).astype(NP_H16)
    return hi, lo


def _build_nc(repeats=1):
    gt = mybir.AluOpType.is_gt
    mult = mybir.AluOpType.mult
    sub = mybir.AluOpType.subtract
    add = mybir.AluOpType.add
    Copy = mybir.ActivationFunctionType.Copy

    beta = float(np.float32(BETA))
    inv_beta = float(np.float64(1.0) / np.float64(np.float32(BETA)))

    nc = bacc.Bacc("TRN2", target_bir_lowering=False)
    xT_d = nc.dram_tensor("xT", [NI, BC], F32, kind="ExternalInput")
    w1s_d = nc.dram_tensor("w1s", [NI, NH], F32, kind="ExternalInput")
    b1s_d = nc.dram_tensor("b1s", [1, NH], F32, kind="ExternalInput")
    w2h_d = nc.dram_tensor("w2h", [NH, NO], H16, kind="ExternalInput")
    w2l_d = nc.dram_tensor("w2l", [NH, NO], H16, kind="ExternalInput")
    b2h_d = nc.dram_tensor("b2h", [1, NO], H16, kind="ExternalInput")
    b2l_d = nc.dram_tensor("b2l", [1, NO], H16, kind="ExternalInput")
    # hi/lo fp16 scaled identities: negIs[2t] + negIs[2t+1] ~ -b^-(t+1) * I
    negIs_d = nc.dram_tensor("negIs", [2 * (T - 1) * 128, 128], H16,
                             kind="ExternalInput")
    spk1_d = nc.dram_tensor("spk1", [T, NH, BC], H16, kind="ExternalOutput")
    spk2_d = nc.dram_tensor("spk2", [T, BC, NO], H16, kind="ExternalOutput")

    with TileContext(nc) as tc:
        with (
            tc.tile_pool(name="wpool", bufs=1) as wpool,
            tc.tile_pool(name="spool", bufs=1) as spool,
            tc.tile_pool(name="kpool", bufs=3) as kpool,
            tc.tile_pool(name="zpool", bufs=1, space="PSUM") as zpool,
            tc.tile_pool(name="ps2pool", bufs=2, space="PSUM") as ps2pool,
        ):
            for rep in range(repeats):
                # ---- load weights / inputs ----
                w1s_t = wpool.tile([128, NHC * NH], F32)
                nc.gpsimd.dma_start(
                    out=w1s_t[:].rearrange("p (c n) -> p c n", c=NHC),
                    in_=w1s_d[:, :].rearrange("(c p) n -> p c n", p=128),
                )
                xT_t = wpool.tile([128, NHC * BC], F32)
                nc.gpsimd.dma_start(
                    out=xT_t[:].rearrange("p (c b) -> p c b", c=NHC),
                    in_=xT_d[:, :].rearrange("(c p) b -> p c b", p=128),
                )
                b1s_t = wpool.tile([1, NH], F32)
                nc.gpsimd.dma_start(out=b1s_t[:], in_=b1s_d[:, :])
                w2h_t = wpool.tile([128, NHC * NO], H16)
                nc.gpsimd.dma_start(
                    out=w2h_t[:].rearrange("p (c o) -> p c o", c=NHC),
                    in_=w2h_d[:, :].rearrange("(c p) o -> p c o", p=128),
                )
                w2l_t = wpool.tile([128, NHC * NO], H16)
                nc.gpsimd.dma_start(
                    out=w2l_t[:].rearrange("p (c o) -> p c o", c=NHC),
                    in_=w2l_d[:, :].rearrange("(c p) o -> p c o", p=128),
                )
                b2h_t = wpool.tile([1, NO], H16)
                nc.gpsimd.dma_start(out=b2h_t[:], in_=b2h_d[:, :])
                b2l_t = wpool.tile([1, NO], H16)
                nc.gpsimd.dma_start(out=b2l_t[:], in_=b2l_d[:, :])
                negIs_t = wpool.tile([128, 2 * (T - 1) * 128], H16)
                nc.gpsimd.dma_start(
                    out=negIs_t[:].rearrange("p (n m) -> p n m", m=128),
                    in_=negIs_d[:, :].rearrange("(n p) m -> p n m", p=128),
                )
                ones_f = wpool.tile([1, BC], F32)
                nc.vector.memset(ones_f[:], 1.0)
                ones_b = wpool.tile([1, 128], H16)
                nc.vector.memset(ones_b[:], 1.0)

                # ---- persistent state ----
                z_ps = zpool.tile([128, NHC * BC], F32)  # 4 PSUM banks
                thr_t = spool.tile([128, NHC * BC], F32)
                m2_t = spool.tile([128, 2 * NO], F32)
                nc.vector.memset(m2_t[:], 0.0)
                spk2_zero = spool.tile([128, 2 * NO], H16)
                nc.vector.memset(spk2_zero[:], 0.0)
                spk2_all = spool.tile([128, T * 2 * NO], H16)

                # ---- fc1 into z PSUM: z_0 = -(b/(1-b)) * (x @ w1.T + b1) ----
                # start=True only on the first matmul of each BANK (even h):
                # it clears has_written for the whole bank; the odd-h region's
                # first matmul then overwrites-and-sets via clear bits. After
                # fc1 every bank has all bits set, so per-step accumulating
                # matmuls add onto ACT-scaled values.
                for h in range(NHC):
                    zslice = z_ps[:, h * BC : (h + 1) * BC]
                    for c in range(NHC):
                        nc.tensor.matmul(
                            zslice,
                            lhsT=w1s_t[:, c * NH + h * 128 : c * NH + (h + 1) * 128],
                            rhs=xT_t[:, c * BC : (c + 1) * BC],
                            start=(h % 2 == 0 and c == 0),
                            stop=False,
                            skip_group_check=True,
                        )
                    nc.tensor.matmul(
                        zslice,
                        lhsT=b1s_t[:, h * 128 : (h + 1) * 128],
                        rhs=ones_f[:],
                        start=False,
                        stop=(h % 2 == 1),
                        skip_group_check=True,
                    )
                    nc.scalar.activation(
                        thr_t[:, h * BC : (h + 1) * BC],
                        zslice,
                        Copy,
                        bias=1.0,
                        scale=inv_beta,
                    )

                # ---- temporal loop ----
                beta64 = np.float64(np.float32(BETA))
                spk2_prev = spk2_zero[:]
                for t in range(T):
                    cmp_scale = float(np.float32(beta64**t))
                    spk_t = kpool.tile([128, NHC * BC], H16, tag="spk")
                    for k in range(NBANK):
                        sl = slice(k * 512, (k + 1) * 512)
                        nc.vector.scalar_tensor_tensor(
                            out=spk_t[:, sl], in0=z_ps[:, sl], scalar=cmp_scale,
                            in1=thr_t[:, sl], op0=mult, op1=gt,
                        )
                    nc.sync.dma_start(
                        out=spk1_d[t].rearrange("(c p) b -> p c b", p=128),
                        in_=spk_t[:].rearrange("p (c b) -> p c b", b=BC),
                    )
                    if t < T - 1:
                        for hl in range(2):
                            wsl = negIs_t[:, (2 * t + hl) * 128 : (2 * t + hl + 1) * 128]
                            for k in range(NBANK):
                                sl = slice(k * 512, (k + 1) * 512)
                                nc.tensor.matmul(
                                    z_ps[:, sl],
                                    lhsT=wsl,
                                    rhs=spk_t[:, sl],
                                    start=False,
                                    stop=True,
                                    skip_group_check=True,
                                )
                    # fc2: cur2[b, o] per batch-half; spk slices as bf16 weights
                    ps2 = []
                    for half in range(2):
                        p2 = ps2pool.tile([128, NO], F32, tag=f"cur2_{half}")
                        ps2.append(p2)
                        for c in range(NHC):
                            lhs = spk_t[
                                :, c * BC + half * 128 : c * BC + half * 128 + 128
                            ]
                            nc.tensor.matmul(
                                p2[:], lhsT=lhs, rhs=w2h_t[:, c * NO : (c + 1) * NO],
                                start=(c == 0), stop=False,
                            )
                            nc.tensor.matmul(
                                p2[:], lhsT=lhs, rhs=w2l_t[:, c * NO : (c + 1) * NO],
                                start=False, stop=False,
                            )
                        nc.tensor.matmul(
                            p2[:], lhsT=ones_b[:], rhs=b2h_t[:], start=False, stop=False
                        )
                        nc.tensor.matmul(
                            p2[:], lhsT=ones_b[:], rhs=b2l_t[:], start=False, stop=True
                        )
                    # layer-2 LIF update mirroring the reference's float-op
                    # order: m2 = (b*m2 + cur2) - spk2_prev ; spk2 = m2 > 1
                    for half in range(2):
                        sl2 = m2_t[:, half * NO : (half + 1) * NO]
                        nc.vector.scalar_tensor_tensor(
                            out=sl2, in0=sl2, scalar=beta, in1=ps2[half][:],
                            op0=mult, op1=add,
                        )
                    nc.vector.tensor_tensor(
                        out=m2_t[:], in0=m2_t[:], in1=spk2_prev, op=sub
                    )
                    spk2_slice = spk2_all[:, t * 2 * NO : (t + 1) * 2 * NO]
                    nc.vector.tensor_scalar(
                        out=spk2_slice, in0=m2_t[:], scalar1=1.0, scalar2=None, op0=gt
                    )
                    spk2_prev = spk2_slice

                # ---- store spk2: sbuf [p, (t h o)] -> dram [t, (h p), o] ----
                src4 = spk2_all[:].rearrange("p (t h o) -> p t h o", t=T, h=2, o=NO)
                dst4 = spk2_d[:, :, :].rearrange("t (h p) o -> h p t o", h=2)
                for half in range(2):
                    nc.sync.dma_start(out=dst4[half], in_=src4[:, :, half, :])

    nc.finalize()
    return nc


def _get_nc(repeats=1):
    if repeats not in _CACHED:
        _CACHED[repeats] = _build_nc(repeats)
    return _CACHED[repeats]


def _prepare_inmaps(x, w1, b1, w2, b2):
    x = np.ascontiguousarray(np.asarray(x, np.float32))
    w1 = np.asarray(w1, np.float32)
    b1 = np.asarray(b1, np.float32)
    w2 = np.asarray(w2, np.float32)
    b2 = np.asarray(b2, np.float32)

    beta32 = np.float64(np.float32(BETA))
    factor = beta32 / (np.float64(1.0) - beta32)  # b/(1-b) with f32 beta

    w1s = np.ascontiguousarray((-factor * w1.T.astype(np.float64)).astype(np.float32))
    b1s = (-factor * b1.astype(np.float64)).astype(np.float32)[None, :]
    w2h, w2l = _split_hilo(np.ascontiguousarray(w2.T, dtype=np.float32))
    b2h, b2l = _split_hilo(b2)
    b2h, b2l = b2h[None, :], b2l[None, :]

    # hi/lo fp16 split of -beta^-(t+1), embedded on identity diagonals
    eye = np.eye(128, dtype=np.float32)
    blocks = []
    for t in range(T - 1):
        c = -(beta32 ** -(t + 1))
        chi = np.asarray(c, np.float32).astype(NP_H16)
        clo = np.asarray(c - np.float64(chi.astype(np.float32)), np.float32).astype(
            NP_H16
        )
        blocks.append((eye * chi.astype(np.float32)).astype(NP_H16))
        blocks.append((eye * clo.astype(np.float32)).astype(NP_H16))
    negIs = np.ascontiguousarray(np.concatenate(blocks, axis=0))

    in_maps = []
    for c in range(NCORES):
        xT_c = np.ascontiguousarray(x[c * BC : (c + 1) * BC].T)
        in_maps.append(
            {
                "xT": xT_c,
                "w1s": w1s,
                "b1s": b1s,
                "w2h": w2h,
                "w2l": w2l,
                "b2h": b2h,
                "b2l": b2l,
                "negIs": negIs,
            }
        )
    return in_maps


def kernel(x, w1, b1, w2, b2):
    global LAST_RESULTS
    in_maps = _prepare_inmaps(x, w1, b1, w2, b2)
    nc = _get_nc()
    res = bass_utils.run_bass_kernel_spmd(
        nc, in_maps, core_ids=list(range(NCORES)), trace=TRACE
    )
    LAST_RESULTS = res

    spikes1 = np.empty((T, B, NH), np.float32)
    spikes2 = np.empty((T, B, NO), np.float32)
    for c in range(NCORES):
        s1 = np.asarray(res.results[c]["spk1"])  # [T, NH, BC] bf16
        spikes1[:, c * BC : (c + 1) * BC, :] = s1.astype(np.float32).transpose(0, 2, 1)
        s2 = np.asarray(res.results[c]["spk2"])  # [T, BC, NO] bf16
        spikes2[:, c * BC : (c + 1) * BC, :] = s2.astype(np.float32)
    return spikes1, spikes2


# revision 26
# speedup vs baseline: 1.7763x; 1.7763x over previous
"""Trainium2 Bass kernel for EnhancedSNN (2-layer LIF spiking net, 50 steps).

Math:
    cur1 = x @ w1.T + b1                      (loop-invariant)
    m_0 = cur1;  m_t = b*m_{t-1} + cur1 - spk_{t-1};  spk_t = (m_t > 1)
    cur2_t = spk1_t @ w2.T + b2               (layer 2 analogous, cur2 varies)

Layer 1 transforms:
    K = cur1/(1-b), z = m - K   =>  z_t = b*z_{t-1} - spk_{t-1},
        spk_t = (z_t > thr), thr = 1 + z_0/b, z_0 = -(b/(1-b))*cur1.
    Undecayed state w_t = z_t / b^t  =>  w_{t+1} = w_t - b^-(t+1) * spk_t and
        spk_t = ((w_t * b^t) > thr)  -- one fused DVE scalar_tensor_tensor op.
    The subtract runs on TensorE for PSUM-resident banks (accumulating matmul
    with hi/lo fp16 scaled identities, exact to ~2^-23) and on DVE for
    SBUF-resident banks (fused STT: w = (-c * spk) + w). No in-place PSUM RMW
    by ACT/DVE (faults the exec unit); only TensorE accumulates into PSUM.

Layer 2 transform (cur2 varies per step, so fold the decay into the weights):
    u_t = m2_t / b^t  =>  u_t = u_{t-1} + b^-t * mm_t - b^-t * spk2_{t-1},
    where mm_t = spk1_t @ w2.T. u lives in PSUM [NO=10, 256] and the matmuls
    accumulate straight onto it with per-step pre-scaled weights
    w2s_t = b^-t * w2.T (hi/lo fp16, 10-column weight loads). b2 folds into a
    per-(o,t) threshold: m2_t > 1  <=>  u~_t > thr2[o,t] = b^-t - g_t*b2[o],
    g_t = sum_{s<=t} b^-s. Spike is one per-partition-scalar tensor_scalar.

Sharding: data-parallel over batch, 8 cores x 256 batch rows. Weights
replicated. Layout: [neuron on partitions, batch on free]; spikes emitted as
fp16 (0/1 exact), host transposes/casts to f32.
"""

import sys

sys.path.insert(0, "/opt/trn_rl_repo")

import numpy as np

import concourse.bass as bass
import concourse.bacc as bacc
import concourse.mybir as mybir
from concourse.tile import TileContext
from concourse import bass_utils

F32 = mybir.dt.float32
H16 = mybir.dt.float16
NP_H16 = np.float16

T = 50
BETA = 0.9
B, NI, NH, NO = 2048, 1024, 1024, 10
NCORES = 8
BC = B // NCORES  # 256 batch rows per core
NHC = NH // 128  # 8 neuron chunks
NBANK = 4  # z banks, each [128, 512] f32 covering two neuron chunks
N_PSUM_BANKS = 3  # banks 0..N-1 PE-updated in PSUM; rest DVE-updated in SBUF

TRACE = False
LAST_RESULTS = None

_CACHED = {}  # repeats -> built program


def _split_hilo(a32):
    hi = a32.astype(NP_H16)
    lo = (a32 - hi.astype(np.float32)).astype(NP_H16)
    return hi, lo


def _build_nc(repeats=1):
    gt = mybir.AluOpType.is_gt
    mult = mybir.AluOpType.mult
    add = mybir.AluOpType.add
    Copy = mybir.ActivationFunctionType.Copy

    inv_beta = float(np.float64(1.0) / np.float64(np.float32(BETA)))
    beta64 = np.float64(np.float32(BETA))

    nc = bacc.Bacc("TRN2", target_bir_lowering=False)
    xT_d = nc.dram_tensor("xT", [NI, BC], F32, kind="ExternalInput")
    w1s_d = nc.dram_tensor("w1s", [NI, NH], F32, kind="ExternalInput")
    b1s_d = nc.dram_tensor("b1s", [1, NH], F32, kind="ExternalInput")
    # per-step scaled fc2 weights, host pre-permuted to partition-major:
    # w2s[p, ((t*2+hl)*NHC+c)*NO+o] = (b^-t * w2.T)[c*128+p, o] (hi/lo fp16)
    w2s_d = nc.dram_tensor("w2s", [128, T * 2 * NHC * NO], H16,
                           kind="ExternalInput")
    # layer-1 subtract identities (partition-major): hi/lo fp16 -b^-(t+1)*I128
    negIs_d = nc.dram_tensor("negIs", [128, 2 * (T - 1) * 128], H16,
                             kind="ExternalInput")
    # layer-2 subtract identities (partition-major): hi/lo fp16 -b^-t * I10
    i10s_d = nc.dram_tensor("i10s", [NO, 2 * (T - 1) * NO], H16,
                            kind="ExternalInput")
    thr2_d = nc.dram_tensor("thr2", [NO, T], F32, kind="ExternalInput")
    spk1_d = nc.dram_tensor("spk1", [T, NH, BC], H16, kind="ExternalOutput")
    spk2_d = nc.dram_tensor("spk2", [NO, T * BC], H16, kind="ExternalOutput")

    with TileContext(nc) as tc:
        with (
            tc.tile_pool(name="wpool", bufs=1) as wpool,
            tc.tile_pool(name="spool", bufs=1) as spool,
            tc.tile_pool(name="kpool", bufs=3) as kpool,
            tc.tile_pool(name="zpool", bufs=1, space="PSUM") as zpool,
            tc.tile_pool(name="upool", bufs=1, space="PSUM") as upool,
        ):
            for rep in range(repeats):
                # ---- load weights / inputs ----
                w1s_t = wpool.tile([128, NHC * NH], F32)
                nc.gpsimd.dma_start(
                    out=w1s_t[:].rearrange("p (c n) -> p c n", c=NHC),
                    in_=w1s_d[:, :].rearrange("(c p) n -> p c n", p=128),
                )
                xT_t = wpool.tile([128, NHC * BC], F32)
                nc.gpsimd.dma_start(
                    out=xT_t[:].rearrange("p (c b) -> p c b", c=NHC),
                    in_=xT_d[:, :].rearrange("(c p) b -> p c b", p=128),
                )
                b1s_t = wpool.tile([1, NH], F32)
                nc.gpsimd.dma_start(out=b1s_t[:], in_=b1s_d[:, :])
                w2s_t = wpool.tile([128, T * 2 * NHC * NO], H16)
                nc.gpsimd.dma_start(out=w2s_t[:], in_=w2s_d[:, :])
                negIs_t = wpool.tile([128, 2 * (T - 1) * 128], H16)
                nc.gpsimd.dma_start(out=negIs_t[:], in_=negIs_d[:, :])
                i10s_t = wpool.tile([NO, 2 * (T - 1) * NO], H16)
                nc.gpsimd.dma_start(out=i10s_t[:], in_=i10s_d[:, :])
                thr2_t = wpool.tile([NO, T], F32)
                nc.gpsimd.dma_start(out=thr2_t[:], in_=thr2_d[:, :])
                ones_f = wpool.tile([1, BC], F32)
                nc.vector.memset(ones_f[:], 1.0)

                # ---- persistent state (one tile per bank so PE-writes and
                # DVE-reads of different banks never falsely serialize) ----
                zb = [
                    zpool.tile([128, 512], F32, name=f"zb{k}", tag=f"zb{k}")
                    for k in range(N_PSUM_BANKS)
                ]
                wb = [
                    spool.tile([128, 512], F32, name=f"wb{k}", tag=f"wb{k}")
                    for k in range(N_PSUM_BANKS, NBANK)
                ]
                thr_t = spool.tile([128, NHC * BC], F32)
                u_ps = upool.tile([NO, BC], F32)
                spk2_all = spool.tile([NO, T * BC], H16)

                def bank_state(k):
                    if k < N_PSUM_BANKS:
                        return zb[k]
                    return wb[k - N_PSUM_BANKS]

                # ---- fc1 into z PSUM: z_0 = -(b/(1-b)) * (x @ w1.T + b1) ----
                # start=True only on the first matmul of each BANK (even h): it
                # clears has_written for the whole bank; the odd-h region's
                # first matmul overwrites-and-sets via clear bits. After fc1
                # every bank has all bits set, so per-step accumulating
                # matmuls add onto the current values.
                for h in range(NHC):
                    k = h // 2
                    if k < N_PSUM_BANKS:
                        ztile = zb[k]
                    else:
                        ztile = zpool.tile(
                            [128, 512], F32, name="zfc1", tag="zfc1", bufs=2
                        ) if h % 2 == 0 else ztile
                    zslice = ztile[:, (h % 2) * BC : (h % 2 + 1) * BC]
                    for c in range(NHC):
                        nc.tensor.matmul(
                            zslice,
                            lhsT=w1s_t[:, c * NH + h * 128 : c * NH + (h + 1) * 128],
                            rhs=xT_t[:, c * BC : (c + 1) * BC],
                            start=(h % 2 == 0 and c == 0),
                            stop=False,
                            skip_group_check=True,
                        )
                    nc.tensor.matmul(
                        zslice,
                        lhsT=b1s_t[:, h * 128 : (h + 1) * 128],
                        rhs=ones_f[:],
                        start=False,
                        stop=(h % 2 == 1),
                        skip_group_check=True,
                    )
                    nc.scalar.activation(
                        thr_t[:, h * BC : (h + 1) * BC],
                        zslice,
                        Copy,
                        bias=1.0,
                        scale=inv_beta,
                    )
                    # SBUF-resident banks: move w out of PSUM once
                    if k >= N_PSUM_BANKS:
                        nc.scalar.activation(
                            wb[k - N_PSUM_BANKS][:, (h % 2) * BC : (h % 2 + 1) * BC],
                            zslice,
                            Copy,
                        )

                # ---- temporal loop ----
                spk2_prev = None
                for t in range(T):
                    cmp_scale = float(np.float32(beta64**t))
                    spk_t = kpool.tile([128, NHC * BC], H16, tag="spk")
                    for k in range(NBANK):
                        sl = slice(k * 512, (k + 1) * 512)
                        nc.vector.scalar_tensor_tensor(
                            out=spk_t[:, sl], in0=bank_state(k)[:], scalar=cmp_scale,
                            in1=thr_t[:, sl], op0=mult, op1=gt,
                        )
                    nc.sync.dma_start(
                        out=spk1_d[t].rearrange("(c p) b -> p c b", p=128),
                        in_=spk_t[:].rearrange("p (c b) -> p c b", b=BC),
                    )
                    if t < T - 1:
                        neg_c = float(-(beta64 ** -(t + 1)))
                        for k in range(N_PSUM_BANKS):
                            sl = slice(k * 512, (k + 1) * 512)
                            for hl in range(2):
                                wsl = negIs_t[
                                    :, (2 * t + hl) * 128 : (2 * t + hl + 1) * 128
                                ]
                                nc.tensor.matmul(
                                    zb[k][:], lhsT=wsl, rhs=spk_t[:, sl],
                                    start=False, stop=True, skip_group_check=True,
                                )
                        for k in range(N_PSUM_BANKS, NBANK):
                            sl = slice(k * 512, (k + 1) * 512)
                            w = wb[k - N_PSUM_BANKS]
                            nc.vector.scalar_tensor_tensor(
                                out=w[:], in0=spk_t[:, sl], scalar=neg_c,
                                in1=w[:], op0=mult, op1=add,
                            )
                    # layer 2: u += b^-t * (spk @ w2.T) - b^-t * spk2_prev
                    for c in range(NHC):
                        for hl in range(2):
                            idx = (t * 2 + hl) * NHC + c
                            nc.tensor.matmul(
                                u_ps[:],
                                lhsT=w2s_t[:, idx * NO : (idx + 1) * NO],
                                rhs=spk_t[:, c * BC : (c + 1) * BC],
                                start=(t == 0 and c == 0 and hl == 0),
                                stop=False,
                                skip_group_check=True,
                            )
                    if t > 0:
                        for hl in range(2):
                            idx = (t - 1) * 2 + hl
                            nc.tensor.matmul(
                                u_ps[:],
                                lhsT=i10s_t[:, idx * NO : (idx + 1) * NO],
                                rhs=spk2_prev,
                                start=False,
                                stop=True,
                                skip_group_check=True,
                            )
                    spk2_slice = spk2_all[:, t * BC : (t + 1) * BC]
                    nc.vector.tensor_scalar(
                        out=spk2_slice, in0=u_ps[:], scalar1=thr2_t[:, t : t + 1],
                        scalar2=None, op0=gt,
                    )
                    spk2_prev = spk2_slice

                nc.sync.dma_start(out=spk2_d[:, :], in_=spk2_all[:])

    nc.finalize()
    return nc


def _get_nc(repeats=1):
    if repeats not in _CACHED:
        _CACHED[repeats] = _build_nc(repeats)
    return _CACHED[repeats]


def _prepare_inmaps(x, w1, b1, w2, b2):
    x = np.ascontiguousarray(np.asarray(x, np.float32))
    w1 = np.asarray(w1, np.float32)
    b1 = np.asarray(b1, np.float32)
    w2 = np.asarray(w2, np.float32)
    b2 = np.asarray(b2, np.float32)

    beta64 = np.float64(np.float32(BETA))
    factor = beta64 / (np.float64(1.0) - beta64)  # b/(1-b) with f32 beta

    w1s = np.ascontiguousarray((-factor * w1.T.astype(np.float64)).astype(np.float32))
    b1s = (-factor * b1.astype(np.float64)).astype(np.float32)[None, :]

    w2T = np.ascontiguousarray(w2.T, dtype=np.float32)  # [NH, NO]
    w2s = np.empty((T, 2, NH, NO), NP_H16)
    for t in range(T):
        scaled = ((beta64 ** -t) * w2T.astype(np.float64)).astype(np.float32)
        hi, lo = _split_hilo(scaled)
        w2s[t, 0], w2s[t, 1] = hi, lo
    # [T,2,NH,NO] -> [128, T*2*NHC*NO] partition-major
    w2s = np.ascontiguousarray(
        w2s.reshape(T, 2, NHC, 128, NO).transpose(3, 0, 1, 2, 4).reshape(128, -1)
    )

    eye = np.eye(128, dtype=np.float32)
    blocks = []
    for t in range(T - 1):
        c = -(beta64 ** -(t + 1))
        chi = np.asarray(c, np.float32).astype(NP_H16)
        clo = np.asarray(c - np.float64(chi.astype(np.float32)), np.float32).astype(
            NP_H16
        )
        blocks.append((eye * chi.astype(np.float32)).astype(NP_H16))
        blocks.append((eye * clo.astype(np.float32)).astype(NP_H16))
    negIs = np.ascontiguousarray(
        np.stack(blocks).transpose(1, 0, 2).reshape(128, -1)
    )

    eye10 = np.eye(NO, dtype=np.float32)
    blocks = []
    for t in range(1, T):
        c = -(beta64 ** -t)
        chi = np.asarray(c, np.float32).astype(NP_H16)
        clo = np.asarray(c - np.float64(chi.astype(np.float32)), np.float32).astype(
            NP_H16
        )
        blocks.append((eye10 * chi.astype(np.float32)).astype(NP_H16))
        blocks.append((eye10 * clo.astype(np.float32)).astype(NP_H16))
    i10s = np.ascontiguousarray(
        np.stack(blocks).transpose(1, 0, 2).reshape(NO, -1)
    )

    # thr2[o, t] = b^-t - g_t * b2[o],  g_t = sum_{s=0..t} b^-s
    thr2 = np.empty((NO, T), np.float32)
    g = np.float64(0.0)
    for t in range(T):
        g += beta64 ** -t
        thr2[:, t] = ((beta64 ** -t) - g * b2.astype(np.float64)).astype(np.float32)

    in_maps = []
    for c in range(NCORES):
        xT_c = np.ascontiguousarray(x[c * BC : (c + 1) * BC].T)
        in_maps.append(
            {
                "xT": xT_c,
                "w1s": w1s,
                "b1s": b1s,
                "w2s": w2s,
                "negIs": negIs,
                "i10s": i10s,
                "thr2": thr2,
            }
        )
    return in_maps


def kernel(x, w1, b1, w2, b2):
    global LAST_RESULTS
    in_maps = _prepare_inmaps(x, w1, b1, w2, b2)
    nc = _get_nc()
    res = bass_utils.run_bass_kernel_spmd(
        nc, in_maps, core_ids=list(range(NCORES)), trace=TRACE
    )
    LAST_RESULTS = res

    spikes1 = np.empty((T, B, NH), np.float32)
    spikes2 = np.empty((T, B, NO), np.float32)
    for c in range(NCORES):
        s1 = np.asarray(res.results[c]["spk1"])  # [T, NH, BC] fp16
        spikes1[:, c * BC : (c + 1) * BC, :] = s1.astype(np.float32).transpose(0, 2, 1)
        s2 = np.asarray(res.results[c]["spk2"])  # [NO, T*BC] fp16
        spikes2[:, c * BC : (c + 1) * BC, :] = (
            s2.reshape(NO, T, BC).transpose(1, 2, 0).astype(np.float32)
        )
    return spikes1, spikes2
